# revision 4
# baseline (speedup 1.0000x reference)
"""Trainium2 Bass kernel for additive attention (Bahdanau), data-parallel over batch.

Problem shapes: B=32, S=4096, H=512, A=256, f32 inputs.
  enc_p = enc @ W_enc + b_enc                    [B,S,A]
  dec_p = dec @ W_dec + b_dec                    [B,A]
  scores = tanh(enc_p + dec_p) @ W_e + b_e       [B,S]
  attn = softmax(scores, axis=1)                 [B,S]
  context = attn-weighted sum of enc             [B,H]
Returns (context, attn).

Sharding: B across 8 cores (4 batches/core); weights replicated. No collectives.

Per-core dataflow (bf16 compute, f32 accumulation):
  1. SWDGE DMA loads enc[b] with inline f32->bf16 cast, natural layout
     [128 s-partitions, 32 s-subtiles, 512 h].
  2. TensorE transposes 128x128 subtiles (bf16) -> PSUM, DVE copies to SBUF
     giving encT [128 h, 4 h-tiles, 512 s] per 512-row chunk.
  3. Projection: psum[a_half, s] += W_enc[h,a_half].T @ encT, K=h in 4 tiles.
  4. ScalarE tanh with per-partition bias (b_enc + dec_p[b]) -> u bf16 [a, s].
  5. Score transposed: psum_sT[s,col] += u[:,s-tile].T @ W_e  (s on partitions).
  6. ScalarE exp(score + b_e) -> p bf16 [128, 32].
  7. Denominator replicated on all partitions via ones.T @ p, free-axis
     reduce, reciprocal.
  8. Context: psum[1, h] += p[:,col].T @ enc_nat[:,col,:] accumulated over s.
  9. Scale by 1/denom, DMA out.

No softmax max-subtraction needed: |score| <= sum|W_e| + |b_e| <= 17, exp is
safe in f32.
"""

import sys

if "/opt/trn_rl_repo" not in sys.path:
    sys.path.insert(0, "/opt/trn_rl_repo")

import numpy as np
import ml_dtypes

import concourse.bass as bass
import concourse.bacc as bacc
import concourse.tile as tile
import concourse.mybir as mybir
from concourse import bass2jax

BF16 = mybir.dt.bfloat16
F32 = mybir.dt.float32
AF = mybir.ActivationFunctionType
NP_BF16 = ml_dtypes.bfloat16

N_CORES = 8
B_FULL, S, H, A = 32, 4096, 512, 256
B = B_FULL // N_CORES          # 4 batches per core
P = 128
N_SUB = S // P                 # 32 s-subtiles per batch
CHUNK = 512                    # s rows per compute chunk
N_CHUNK = S // CHUNK           # 8 chunks per batch
SS = CHUNK // P                # 4 s-subtiles per chunk
KH = H // P                    # 4 k-tiles over h
AH = A // P                    # 2 a-halves


def build_bass(reps=1):
    nc = bacc.Bacc(
        "TRN2",
        target_bir_lowering=False,
        debug=False,
        enable_asserts=False,
        num_devices=N_CORES,
    )

    enc = nc.dram_tensor("enc", [B, S, H], F32, kind="ExternalInput").ap()
    wenc = nc.dram_tensor("wenc", [H, A], BF16, kind="ExternalInput").ap()
    wdec = nc.dram_tensor("wdec", [H, A], BF16, kind="ExternalInput").ap()
    decT = nc.dram_tensor("decT", [H, B], BF16, kind="ExternalInput").ap()
    we = nc.dram_tensor("we", [A, 1], BF16, kind="ExternalInput").ap()
    bias_ed = nc.dram_tensor("bias_ed", [P, AH], F32, kind="ExternalInput").ap()
    bec = nc.dram_tensor("bec", [P, 1], F32, kind="ExternalInput").ap()
    ident = nc.dram_tensor("ident", [P, P], BF16, kind="ExternalInput").ap()
    ones = nc.dram_tensor("ones", [P, P], BF16, kind="ExternalInput").ap()

    out_ctx = nc.dram_tensor("out_ctx", [B, H], F32, kind="ExternalOutput").ap()
    out_attn = nc.dram_tensor("out_attn", [B, S], F32, kind="ExternalOutput").ap()

    with tile.TileContext(nc) as tc:
        with (
            tc.tile_pool(name="consts", bufs=1) as cpool,
            tc.tile_pool(name="enc_nat", bufs=2) as enc_pool,
            tc.tile_pool(name="encT", bufs=3) as encT_pool,
            tc.tile_pool(name="u", bufs=3) as u_pool,
            tc.tile_pool(name="small", bufs=2) as small_pool,
            tc.tile_pool(name="psum_t", bufs=2, space="PSUM") as pt_pool,
            tc.tile_pool(name="psum_p", bufs=2, space="PSUM") as pp_pool,
            tc.tile_pool(name="psum_sT", bufs=1, space="PSUM") as psT_pool,
            tc.tile_pool(name="psum_ctx", bufs=1, space="PSUM") as pctx_pool,
            tc.tile_pool(name="psum_misc", bufs=1, space="PSUM") as pmisc_pool,
        ):
            # ---- constants ----
            wenc_sb = cpool.tile([P, KH, A], BF16, tag="wenc")
            nc.sync.dma_start(wenc_sb[:], wenc.rearrange("(ko ki) a -> ki ko a", ki=P))
            wdec_sb = cpool.tile([P, KH, A], BF16, tag="wdec")
            nc.sync.dma_start(wdec_sb[:], wdec.rearrange("(ko ki) a -> ki ko a", ki=P))
            decT_sb = cpool.tile([P, KH, B], BF16, tag="decT")
            nc.sync.dma_start(decT_sb[:], decT.rearrange("(ko ki) b -> ki ko b", ki=P))
            we_sb = cpool.tile([P, AH, 1], BF16, tag="we")
            nc.sync.dma_start(we_sb[:], we.rearrange("(ko ki) o -> ki ko o", ki=P))
            bias_ed_sb = cpool.tile([P, AH], F32, tag="bias_ed")
            nc.sync.dma_start(bias_ed_sb[:], bias_ed)
            bec_sb = cpool.tile([P, 1], F32, tag="bec")
            nc.sync.dma_start(bec_sb[:], bec)
            ident_sb = cpool.tile([P, P], BF16, tag="ident")
            nc.sync.dma_start(ident_sb[:], ident)
            ones_sb = cpool.tile([P, P], BF16, tag="ones")
            nc.sync.dma_start(ones_sb[:], ones)

            # ---- decoder projection: bias_sb[:, half*B+b] = b_enc+b_dec+dec_p ----
            psum_dp = pmisc_pool.tile([P, AH * B], F32, tag="misc")
            for half in range(AH):
                for k in range(KH):
                    nc.tensor.matmul(
                        psum_dp[:, half * B:(half + 1) * B],
                        lhsT=wdec_sb[:, k, half * P:(half + 1) * P],
                        rhs=decT_sb[:, k, :],
                        start=(k == 0),
                        stop=(k == KH - 1),
                    )
            bias_sb = cpool.tile([P, AH * B], F32, tag="bias_sb")
            for half in range(AH):
                nc.scalar.activation(
                    bias_sb[:, half * B:(half + 1) * B],
                    psum_dp[:, half * B:(half + 1) * B],
                    AF.Identity,
                    bias=bias_ed_sb[:, half:half + 1],
                )

            # ---- main loop over local batches ----
            for b in [bb for _ in range(reps) for bb in range(B)]:
                enc_nat = enc_pool.tile([P, N_SUB, H], BF16, tag="enc_nat")
                for c in range(N_CHUNK):
                    nc.gpsimd.dma_start(
                        enc_nat[:, c * SS:(c + 1) * SS, :],
                        enc[b, c * CHUNK:(c + 1) * CHUNK, :].rearrange(
                            "(ss p) h -> p ss h", p=P
                        ),
                    )

                psum_sT = psT_pool.tile([P, N_SUB], F32, tag="sT")

                for c in range(N_CHUNK):
                    # transpose chunk: encT[h, hj, s-rel]
                    encT = encT_pool.tile([P, KH, CHUNK], BF16, tag="encT")
                    for hj in range(KH):
                        psum_t = pt_pool.tile([P, CHUNK], BF16, tag="pt")
                        for ss in range(SS):
                            nc.tensor.transpose(
                                psum_t[:, ss * P:(ss + 1) * P],
                                enc_nat[:, c * SS + ss, hj * P:(hj + 1) * P],
                                ident_sb[:],
                            )
                        nc.vector.tensor_copy(encT[:, hj, :], psum_t[:])

                    # projection + tanh -> u[a_rel, half, s-rel]
                    u = u_pool.tile([P, AH, CHUNK], BF16, tag="u")
                    for half in range(AH):
                        psum_p = pp_pool.tile([P, CHUNK], F32, tag="pp")
                        for k in range(KH):
                            nc.tensor.matmul(
                                psum_p[:],
                                lhsT=wenc_sb[:, k, half * P:(half + 1) * P],
                                rhs=encT[:, k, :],
                                start=(k == 0),
                                stop=(k == KH - 1),
                            )
                        nc.scalar.activation(
                            u[:, half, :],
                            psum_p[:],
                            AF.Tanh,
                            bias=bias_sb[:, half * B + b:half * B + b + 1],
                        )

                    # scores, transposed: psum_sT[s_rel, col]
                    for ss in range(SS):
                        col = c * SS + ss
                        for half in range(AH):
                            nc.tensor.matmul(
                                psum_sT[:, col:col + 1],
                                lhsT=u[:, half, ss * P:(ss + 1) * P],
                                rhs=we_sb[:, half, :],
                                start=(half == 0),
                                stop=(half == AH - 1),
                            )

                # p = exp(score + b_e), bf16 [128, 32]
                p_T = small_pool.tile([P, N_SUB], BF16, tag="p_T")
                nc.scalar.activation(p_T[:], psum_sT[:], AF.Exp, bias=bec_sb[:])

                # denom replicated on all partitions
                psum_dr = pmisc_pool.tile([P, N_SUB], F32, tag="misc")
                nc.tensor.matmul(psum_dr[:], lhsT=ones_sb[:], rhs=p_T[:],
                                 start=True, stop=True)
                denr = small_pool.tile([P, 1], F32, tag="denr")
                nc.vector.reduce_sum(denr[:], psum_dr[:], axis=mybir.AxisListType.X)
                recip = small_pool.tile([P, 1], F32, tag="recip")
                nc.vector.reciprocal(recip[:], denr[:])

                # context accumulation over all 32 s-subtiles
                psum_ctx = pctx_pool.tile([1, H], F32, tag="ctx")
                for col in range(N_SUB):
                    nc.tensor.matmul(
                        psum_ctx[:],
                        lhsT=p_T[:, col:col + 1],
                        rhs=enc_nat[:, col, :],
                        start=(col == 0),
                        stop=(col == N_SUB - 1),
                    )

                # outputs
                attn_sb = small_pool.tile([P, N_SUB], F32, tag="attn_sb")
                nc.vector.tensor_scalar_mul(attn_sb[:], p_T[:], recip[:])
                nc.sync.dma_start(
                    out_attn[b].rearrange("(col p) -> p col", p=P), attn_sb[:]
                )
                ctx_sb = small_pool.tile([1, H], F32, tag="ctx_sb")
                nc.vector.tensor_scalar_mul(ctx_sb[:], psum_ctx[:], recip[0:1, :])
                nc.sync.dma_start(out_ctx[b:b + 1, :], ctx_sb[:])

    nc.compile()
    return nc


def make_in_maps(encoder_outputs, decoder_hidden, W_enc, b_enc, W_dec, b_dec,
                 W_e, b_e):
    enc = np.asarray(encoder_outputs, np.float32)
    dec = np.asarray(decoder_hidden, np.float32)
    W_enc = np.asarray(W_enc, np.float32)
    b_enc = np.asarray(b_enc, np.float32)
    W_dec = np.asarray(W_dec, np.float32)
    b_dec = np.asarray(b_dec, np.float32)
    W_e = np.asarray(W_e, np.float32)
    b_e = float(np.asarray(b_e))

    wenc_bf = W_enc.astype(NP_BF16)
    wdec_bf = W_dec.astype(NP_BF16)
    decT_bf = np.ascontiguousarray(dec.T).astype(NP_BF16)      # [H, 32]
    we_col = W_e.reshape(A, 1).astype(NP_BF16)
    bias_ed = np.ascontiguousarray((b_enc + b_dec).reshape(AH, P).T).astype(
        np.float32)                                            # [128, 2]
    bec = np.full((P, 1), b_e, np.float32)
    ident_np = np.eye(P, dtype=NP_BF16)
    ones_np = np.ones((P, P), dtype=NP_BF16)

    in_maps = []
    for i in range(N_CORES):
        in_maps.append({
            "enc": enc[i * B:(i + 1) * B],
            "wenc": wenc_bf,
            "wdec": wdec_bf,
            "decT": np.ascontiguousarray(decT_bf[:, i * B:(i + 1) * B]),
            "we": we_col,
            "bias_ed": bias_ed,
            "bec": bec,
            "ident": ident_np,
            "ones": ones_np,
        })
    return in_maps


_NC_CACHE = []


def get_nc():
    if not _NC_CACHE:
        _NC_CACHE.append(build_bass())
    return _NC_CACHE[0]


def kernel(encoder_outputs, decoder_hidden, W_enc, b_enc, W_dec, b_dec, W_e,
           b_e):
    nc = get_nc()
    in_maps = make_in_maps(encoder_outputs, decoder_hidden, W_enc, b_enc,
                           W_dec, b_dec, W_e, b_e)
    results = bass2jax.run_bass_via_pjrt(nc, in_maps, n_cores=N_CORES)
    context = np.concatenate([r["out_ctx"] for r in results], axis=0)
    attn = np.concatenate([r["out_attn"] for r in results], axis=0)
    return context, attn


# revision 17
# speedup vs baseline: 15.2172x; 15.2172x over previous
"""Trainium2 Bass kernel for additive attention (Bahdanau), data-parallel over batch.

Problem shapes: B=32, S=4096, H=512, A=256, f32 inputs.
  enc_p = enc @ W_enc + b_enc                    [B,S,A]
  dec_p = dec @ W_dec + b_dec                    [B,A]
  scores = tanh(enc_p + dec_p) @ W_e + b_e       [B,S]
  attn = softmax(scores, axis=1)                 [B,S]
  context = attn-weighted sum of enc             [B,H]
Returns (context, attn).

Sharding: B across 8 cores (4 batches/core); weights replicated. No collectives.

Per-core dataflow (bf16 compute, f32 accumulation):
  1. SWDGE DMA loads enc[b] with inline f32->bf16 cast, natural layout
     [128 s-partitions, 32 s-subtiles, 512 h].
  2. TensorE transposes 128x128 subtiles (bf16) -> PSUM, DVE copies to SBUF
     giving encT [128 h, 4 h-tiles, 512 s] per 512-row chunk.
  3. Projection: psum[a_half, s] += W_enc[h,a_half].T @ encT, K=h in 4 tiles.
  4. ScalarE tanh with per-partition bias (b_enc + dec_p[b]) -> u bf16 [a, s].
  5. Score transposed: psum_sT[s,col] += u[:,s-tile].T @ W_e  (s on partitions).
  6. ScalarE exp(score + b_e) -> p bf16 [128, 32].
  7. Denominator replicated on all partitions via ones.T @ p, free-axis
     reduce, reciprocal.
  8. Context: psum[1, h] += p[:,col].T @ enc_nat[:,col,:] accumulated over s.
  9. Scale by 1/denom, DMA out.

No softmax max-subtraction needed: |score| <= sum|W_e| + |b_e| <= 17, exp is
safe in f32.
"""

import sys

if "/opt/trn_rl_repo" not in sys.path:
    sys.path.insert(0, "/opt/trn_rl_repo")

import numpy as np
import ml_dtypes

import concourse.bass as bass
import concourse.bacc as bacc
import concourse.tile as tile
import concourse.mybir as mybir
from concourse import bass2jax

BF16 = mybir.dt.bfloat16
F32 = mybir.dt.float32
AF = mybir.ActivationFunctionType
NP_BF16 = ml_dtypes.bfloat16

N_CORES = 8
B_FULL, S, H, A = 32, 4096, 512, 256
B = B_FULL // N_CORES          # 4 batches per core
P = 128
N_SUB = S // P                 # 32 s-subtiles per batch
CHUNK = 512                    # s rows per compute chunk
N_CHUNK = S // CHUNK           # 8 chunks per batch
SS = CHUNK // P                # 4 s-subtiles per chunk
KH = H // P                    # 4 k-tiles over h
AH = A // P                    # 2 a-halves


def build_bass(reps=1):
    nc = bacc.Bacc(
        "TRN2",
        target_bir_lowering=False,
        debug=False,
        enable_asserts=False,
        num_devices=N_CORES,
    )

    enc = nc.dram_tensor("enc", [B, S, H], F32, kind="ExternalInput").ap()
    wenc = nc.dram_tensor("wenc", [H, A], BF16, kind="ExternalInput").ap()
    wdec = nc.dram_tensor("wdec", [H, A], BF16, kind="ExternalInput").ap()
    decT = nc.dram_tensor("decT", [H, B], BF16, kind="ExternalInput").ap()
    we = nc.dram_tensor("we", [A, 1], BF16, kind="ExternalInput").ap()
    bias_ed = nc.dram_tensor("bias_ed", [P, AH], F32, kind="ExternalInput").ap()
    bec = nc.dram_tensor("bec", [P, 1], F32, kind="ExternalInput").ap()
    ident = nc.dram_tensor("ident", [P, P], BF16, kind="ExternalInput").ap()
    ones = nc.dram_tensor("ones", [P, P], BF16, kind="ExternalInput").ap()

    out_ctx = nc.dram_tensor("out_ctx", [B, H], F32, kind="ExternalOutput").ap()
    out_attn = nc.dram_tensor("out_attn", [B, S], F32, kind="ExternalOutput").ap()

    with tile.TileContext(nc) as tc:
        with (
            tc.tile_pool(name="consts", bufs=1) as cpool,
            tc.tile_pool(name="enc_nat", bufs=3) as enc_pool,
            tc.tile_pool(name="encT", bufs=3) as encT_pool,
            tc.tile_pool(name="u", bufs=3) as u_pool,
            tc.tile_pool(name="small", bufs=2) as small_pool,
            tc.tile_pool(name="dram", bufs=2, space="DRAM") as dram_pool,
            tc.tile_pool(name="psum_t", bufs=3, space="PSUM") as pt_pool,
            tc.tile_pool(name="psum_p", bufs=3, space="PSUM") as pp_pool,
            tc.tile_pool(name="psum_sc", bufs=1, space="PSUM") as psc_pool,
            tc.tile_pool(name="psum_ctx", bufs=1, space="PSUM") as pctx_pool,
        ):
            # ---- constants ----
            wenc_sb = cpool.tile([P, KH, A], BF16, tag="wenc")
            nc.sync.dma_start(wenc_sb[:], wenc.rearrange("(ko ki) a -> ki ko a", ki=P))
            wdec_sb = cpool.tile([P, KH, A], BF16, tag="wdec")
            nc.sync.dma_start(wdec_sb[:], wdec.rearrange("(ko ki) a -> ki ko a", ki=P))
            decT_sb = cpool.tile([P, KH, B], BF16, tag="decT")
            nc.sync.dma_start(decT_sb[:], decT.rearrange("(ko ki) b -> ki ko b", ki=P))
            we_sb = cpool.tile([P, AH, 1], BF16, tag="we")
            nc.sync.dma_start(we_sb[:], we.rearrange("(ko ki) o -> ki ko o", ki=P))
            bias_ed_sb = cpool.tile([P, AH], F32, tag="bias_ed")
            nc.sync.dma_start(bias_ed_sb[:], bias_ed)
            bec_sb = cpool.tile([P, 1], F32, tag="bec")
            nc.sync.dma_start(bec_sb[:], bec)
            ident_sb = cpool.tile([P, P], BF16, tag="ident")
            nc.sync.dma_start(ident_sb[:], ident)
            ones_sb = cpool.tile([P, P], BF16, tag="ones")
            nc.sync.dma_start(ones_sb[:], ones)

            # ---- decoder projection: bias_sb[:, half*B+b] = b_enc+b_dec+dec_p ----
            psum_dp = psc_pool.tile([P, AH * B], F32, tag="sc")
            for half in range(AH):
                for k in range(KH):
                    nc.tensor.matmul(
                        psum_dp[:, half * B:(half + 1) * B],
                        lhsT=wdec_sb[:, k, half * P:(half + 1) * P],
                        rhs=decT_sb[:, k, :],
                        start=(k == 0),
                        stop=(k == KH - 1),
                    )
            bias_sb = cpool.tile([P, AH * B], F32, tag="bias_sb")
            for half in range(AH):
                nc.scalar.activation(
                    bias_sb[:, half * B:(half + 1) * B],
                    psum_dp[:, half * B:(half + 1) * B],
                    AF.Identity,
                    bias=bias_ed_sb[:, half:half + 1],
                )

            # ---- main loop over local batches ----
            first = True
            for b in [bb for _ in range(reps) for bb in range(B)]:
                enc_nat = enc_pool.tile([P, N_SUB, H], BF16, tag="enc_nat")
                # batch 0: fine-grained DMAs so the pipeline starts early;
                # later batches: 2 big DMAs (prefetched during prior batch)
                if first:
                    # fine-grained DMAs so the pipeline starts early
                    bounds = [0, 1, 2, 3] + list(range(SS, N_SUB + 1, SS))
                else:
                    bounds = [0, N_SUB // 2, N_SUB]
                for lo, hi in zip(bounds[:-1], bounds[1:]):
                    nc.gpsimd.dma_start(
                        enc_nat[:, lo:hi, :],
                        enc[b, lo * P:hi * P, :].rearrange(
                            "(ss p) h -> p ss h", p=P
                        ),
                    )
                first = False

                psum_sT = psc_pool.tile([P, N_SUB], F32, tag="sc")

                for c in range(N_CHUNK):
                    # transpose chunk: encT[h, hj, s-rel]; two h-tiles per
                    # PSUM bank so one DVE copy moves 2*128x512
                    encT = encT_pool.tile([P, KH, CHUNK], BF16, tag="encT")
                    for hp in range(KH // 2):
                        psum_t = pt_pool.tile([P, 2 * CHUNK], BF16, tag="pt")
                        for hj2 in range(2):
                            hj = hp * 2 + hj2
                            for ss in range(SS):
                                nc.tensor.transpose(
                                    psum_t[:, hj2 * CHUNK + ss * P:
                                           hj2 * CHUNK + (ss + 1) * P],
                                    enc_nat[:, c * SS + ss, hj * P:(hj + 1) * P],
                                    ident_sb[:],
                                )
                        nc.vector.tensor_copy(
                            encT[:, hp * 2:(hp + 1) * 2, :], psum_t[:]
                        )

                    # projection + tanh -> u[a_rel, half, s-rel]
                    u = u_pool.tile([P, AH, CHUNK], BF16, tag="u")
                    for half in range(AH):
                        psum_p = pp_pool.tile([P, CHUNK], F32, tag="pp")
                        for k in range(KH):
                            nc.tensor.matmul(
                                psum_p[:],
                                lhsT=wenc_sb[:, k, half * P:(half + 1) * P],
                                rhs=encT[:, k, :],
                                start=(k == 0),
                                stop=(k == KH - 1),
                            )
                        nc.scalar.activation(
                            u[:, half, :],
                            psum_p[:],
                            AF.Tanh,
                            bias=bias_sb[:, half * B + b:half * B + b + 1],
                        )

                    # scores, transposed: psum_sT[s_rel, col]
                    for ss in range(SS):
                        col = c * SS + ss
                        for half in range(AH):
                            nc.tensor.matmul(
                                psum_sT[:, col:col + 1],
                                lhsT=u[:, half, ss * P:(ss + 1) * P],
                                rhs=we_sb[:, half, :],
                                start=(half == 0),
                                stop=(half == AH - 1),
                            )

                # p = exp(score + b_e), bf16 [128, 32]
                p_T = small_pool.tile([P, N_SUB], BF16, tag="p_T")
                nc.scalar.activation(p_T[:], psum_sT[:], AF.Exp, bias=bec_sb[:])

                # denom replicated on all partitions
                psum_dr = psc_pool.tile([P, N_SUB], F32, tag="sc")
                nc.tensor.matmul(psum_dr[:], lhsT=ones_sb[:], rhs=p_T[:],
                                 start=True, stop=True)
                denr = small_pool.tile([P, 1], F32, tag="denr")
                nc.vector.reduce_sum(denr[:], psum_dr[:], axis=mybir.AxisListType.X)
                recip = small_pool.tile([P, 1], F32, tag="recip")
                nc.vector.reciprocal(recip[:], denr[:])

                # context accumulation over all 32 s-subtiles
                psum_ctx = pctx_pool.tile([1, H], F32, tag="ctx")
                for col in range(N_SUB):
                    nc.tensor.matmul(
                        psum_ctx[:],
                        lhsT=p_T[:, col:col + 1],
                        rhs=enc_nat[:, col, :],
                        start=(col == 0),
                        stop=(col == N_SUB - 1),
                    )

                # outputs
                attn_sb = small_pool.tile([P, N_SUB], F32, tag="attn_sb")
                nc.vector.tensor_scalar_mul(attn_sb[:], p_T[:], recip[:])
                nc.sync.dma_start(
                    out_attn[b].rearrange("(col p) -> p col", p=P), attn_sb[:]
                )
                ctx_sb = small_pool.tile([1, H], F32, tag="ctx_sb")
                nc.vector.tensor_scalar_mul(ctx_sb[:], psum_ctx[:], recip[0:1, :])
                nc.sync.dma_start(out_ctx[b:b + 1, :], ctx_sb[:])

    nc.compile()
    return nc


def make_in_maps(encoder_outputs, decoder_hidden, W_enc, b_enc, W_dec, b_dec,
                 W_e, b_e):
    enc = np.asarray(encoder_outputs, np.float32)
    dec = np.asarray(decoder_hidden, np.float32)
    W_enc = np.asarray(W_enc, np.float32)
    b_enc = np.asarray(b_enc, np.float32)
    W_dec = np.asarray(W_dec, np.float32)
    b_dec = np.asarray(b_dec, np.float32)
    W_e = np.asarray(W_e, np.float32)
    b_e = float(np.asarray(b_e))

    wenc_bf = W_enc.astype(NP_BF16)
    wdec_bf = W_dec.astype(NP_BF16)
    decT_bf = np.ascontiguousarray(dec.T).astype(NP_BF16)      # [H, 32]
    we_col = W_e.reshape(A, 1).astype(NP_BF16)
    bias_ed = np.ascontiguousarray((b_enc + b_dec).reshape(AH, P).T).astype(
        np.float32)                                            # [128, 2]
    bec = np.full((P, 1), b_e, np.float32)
    ident_np = np.eye(P, dtype=NP_BF16)
    ones_np = np.ones((P, P), dtype=NP_BF16)

    in_maps = []
    for i in range(N_CORES):
        in_maps.append({
            "enc": enc[i * B:(i + 1) * B],
            "wenc": wenc_bf,
            "wdec": wdec_bf,
            "decT": np.ascontiguousarray(decT_bf[:, i * B:(i + 1) * B]),
            "we": we_col,
            "bias_ed": bias_ed,
            "bec": bec,
            "ident": ident_np,
            "ones": ones_np,
        })
    return in_maps


_NC_CACHE = []
_RUNNER_CACHE = []


def get_nc():
    if not _NC_CACHE:
        _NC_CACHE.append(build_bass())
    return _NC_CACHE[0]


def _get_runner():
    """Jitted shard_map runner, built once so repeated kernel() calls only
    pay input transfer + execution."""
    if _RUNNER_CACHE:
        return _RUNNER_CACHE[0]
    import jax
    from jax.sharding import Mesh, PartitionSpec
    from jax.experimental.shard_map import shard_map

    nc = get_nc()
    bass2jax.install_neuronx_cc_hook()
    partition_name = (nc.partition_id_tensor.name
                      if nc.partition_id_tensor else None)
    in_names, out_names, out_avals, zero_outs = [], [], [], []
    for alloc in nc.m.functions[0].allocations:
        if not isinstance(alloc, mybir.MemoryLocationSet):
            continue
        name = alloc.memorylocations[0].name
        if alloc.kind == "ExternalInput":
            if name != partition_name:
                in_names.append(name)
        elif alloc.kind == "ExternalOutput":
            out_names.append(name)
            shape = tuple(alloc.tensor_shape)
            dtype = mybir.dt.np(alloc.dtype)
            out_avals.append(jax.core.ShapedArray(shape, dtype))
            zero_outs.append(np.zeros(shape, dtype))
    n_params = len(in_names)
    all_names = list(in_names) + out_names
    if partition_name is not None:
        all_names.append(partition_name)

    def _body(*args):
        operands = list(args)
        if partition_name is not None:
            operands.append(bass2jax.partition_id_tensor())
        outs = bass2jax._bass_exec_p.bind(
            *operands,
            out_avals=tuple(out_avals),
            in_names=tuple(all_names),
            out_names=tuple(out_names),
            lowering_input_output_aliases=(),
            sim_require_finite=True,
            sim_require_nnan=True,
            nc=nc,
        )
        return tuple(outs)

    devices = jax.devices()[:N_CORES]
    mesh = Mesh(np.asarray(devices), ("core",))
    in_specs = (PartitionSpec("core"),) * (n_params + len(out_avals))
    out_specs = (PartitionSpec("core"),) * len(out_avals)
    fn = jax.jit(
        shard_map(_body, mesh=mesh, in_specs=in_specs, out_specs=out_specs,
                  check_rep=False),
        keep_unused=True,
    )
    runner = (fn, in_names, out_names, zero_outs)
    _RUNNER_CACHE.append(runner)
    return runner


def kernel(encoder_outputs, decoder_hidden, W_enc, b_enc, W_dec, b_dec, W_e,
           b_e):
    in_maps = make_in_maps(encoder_outputs, decoder_hidden, W_enc, b_enc,
                           W_dec, b_dec, W_e, b_e)
    fn, in_names, out_names, zero_outs = _get_runner()
    concat_in = [
        np.concatenate([np.asarray(in_maps[c][nm]) for c in range(N_CORES)],
                       axis=0)
        for nm in in_names
    ]
    concat_zeros = [
        np.zeros((N_CORES * z.shape[0], *z.shape[1:]), z.dtype)
        for z in zero_outs
    ]
    outs = fn(*concat_in, *concat_zeros)
    got = {nm: np.asarray(outs[i]) for i, nm in enumerate(out_names)}
    context = got["out_ctx"].reshape(B_FULL, H)
    attn = got["out_attn"].reshape(B_FULL, S)
    return context, attn
